# revision 24
# baseline (speedup 1.0000x reference)
"""Bass/Tile kernel for nn_Diffeo: horizontal bilinear remap as banded matmul.

v5 (bf16, config-uniform PE bursts): all I/O in bf16 (rel-err budget 2e-2;
measured ~6e-3).  Key perf insight from microbenchmarks: matmuls stream at
N/2.4GHz + 2.5ns ONLY while tile_size config stays constant; alternating
M=128 / M=64 matmuls forces a full pipeline drain per MM (~230ns).  So all
(128,128)-config MMs (t-gen + planes 0-127) are grouped per pair, then all
(128,64)-config MMs (planes 128-191, pair-packed via tile_position).

Per core (H-sharded, 64 rows y, all 192 b*c planes):
  out[bc, y, x_out] = sum_{x_in} imgT[y, x_in, bc] * hat(x_in - xn[y, x_out])

Weight generation per row pair with host-folded block offsets:
  xn'(x) = xn(x) - 128*kown(x)
  t'[p, x] = xn'(x) - p     K=128 matmul: lhsT = c4all slice (rows 4*pr+j
                            select pair pr's bf16-split pieces; other rows 0)
  abs1 (ACT):  a[x]     = |t'|        owned columns [0, 512)
  abs2 (ACT):  a[512+x] = |t' - 128|  accum strip (right-neighbor taps)
  pass2 (DVE): w = min(a - 1, 0) = -hat   (one 4x tensor_scalar, bf16)

Matmuls per (row, plane-half): 4 "own" (start=True, disjoint [B_k, B_k+1))
+ 3 "accum" (start=False, [GS_k, B_k)); accum for block k is issued BEFORE
own_k because own_k's start=True clears the bank's has_written bits.
Output staged 4 rows / 512KB+256KB DMAs; copies negate+cast on ACT/DVE.
"""

import sys
from contextlib import ExitStack

sys.path.insert(0, "/opt/trn_rl_repo")

import numpy as np
import ml_dtypes

import concourse.bass as bass
import concourse.mybir as mybir
import concourse.tile as tile
from concourse import bacc
from concourse._compat import axon_active

F32 = mybir.dt.float32
BF16 = mybir.dt.bfloat16
BF16_NP = ml_dtypes.bfloat16

H = W = 512
NPLANE = 192            # 64 batches * 3 channels
NCORES = 8
YPC = H // NCORES       # 64 rows per core
YG = 8                  # rows per input-DMA group
NG = YPC // YG          # 8 groups
KBLK = 4                # x_in blocks of 128
NPAIR = YPC // 2        # 32 row pairs per core


def compute_windows(xn: np.ndarray):
    """From the full xn field [H, W] (float64), derive the x_out windows."""
    mn = xn.min(axis=0)
    mx = xn.max(axis=0)
    tap_lo = np.floor(mn).astype(int)
    tap_hi = np.floor(mx).astype(int) + 1
    B = [0]
    GS = [0]
    for k in range(1, KBLK):
        lo = 128 * k
        cols_prev = np.nonzero(tap_lo < lo)[0]
        ge_prev = int(cols_prev.max()) + 1 if len(cols_prev) else 0
        ge_prev = min(ge_prev + 2, W)
        cols_k = np.nonzero(tap_hi >= lo)[0]
        gs_k = int(cols_k.min()) if len(cols_k) else W
        gs_k = max(gs_k - 2, 0)
        assert gs_k < ge_prev, f"no overlap at block {k}: {gs_k} {ge_prev}"
        assert ge_prev - 128 * (k - 1) < 256, "band too wide for scheme"
        B.append(ge_prev)
        GS.append(gs_k)
    B.append(W)
    assert all(B[i] < B[i + 1] for i in range(4)), f"bad B {B}"
    for k in range(1, KBLK):
        assert GS[k] >= B[k - 1], f"segment {k} leaks left: {GS[k]} < {B[k-1]}"
    SOFF = GS[1]
    SW = B[3] - SOFF
    SW = SW + (SW % 2)
    return B, GS, SOFF, SW


def build_program(B, GS, SOFF, SW, num_devices: int = NCORES):
    OWNW = 512 + SW
    nc = bacc.Bacc(
        "TRN2",
        target_bir_lowering=False,
        debug=not axon_active(),
        num_devices=num_devices,
    )
    imgT = nc.dram_tensor("imgT", [NG, KBLK, 128, YG * NPLANE], BF16,
                          kind="ExternalInput").ap()
    xn4 = nc.dram_tensor("xn4", [128, 2 * W], BF16, kind="ExternalInput").ap()
    c4a = nc.dram_tensor("c4a", [128, NPAIR * 128], BF16, kind="ExternalInput").ap()
    kb = nc.dram_tensor("kb", [128, 1], F32, kind="ExternalInput").ap()
    out1 = nc.dram_tensor("out1", [128, YPC, W], BF16, kind="ExternalOutput").ap()
    # out2 [h, plane64, pair, x]: per-partition runs are 2KB-contiguous
    out2 = nc.dram_tensor("out2", [2, 64, NPAIR, W], BF16, kind="ExternalOutput").ap()

    with tile.TileContext(nc) as tc, ExitStack() as ctx:
        const_pool = ctx.enter_context(tc.tile_pool(name="const", bufs=1))
        dpool = ctx.enter_context(tc.tile_pool(name="dt", bufs=4))
        tpool = ctx.enter_context(tc.tile_pool(name="psum_t", bufs=1, space="PSUM"))
        opool1 = ctx.enter_context(tc.tile_pool(name="psum_o1", bufs=2, space="PSUM"))
        opool2 = ctx.enter_context(tc.tile_pool(name="psum_o2", bufs=2, space="PSUM"))
        apool = ctx.enter_context(tc.tile_pool(name="abs", bufs=2))
        wpool = ctx.enter_context(tc.tile_pool(name="wt", bufs=3))
        spool1 = ctx.enter_context(tc.tile_pool(name="osb1", bufs=3))
        spool2 = ctx.enter_context(tc.tile_pool(name="osb2", bufs=3))

        xn4_sb = const_pool.tile([128, 2 * W], BF16)
        nc.sync.dma_start(xn4_sb[:], xn4[:])
        kb_sb = const_pool.tile([128, 1], F32)
        nc.sync.dma_start(kb_sb[:], kb[:])
        # split the 1MB c4a load so pair 0's slice lands quickly
        c4a_sb = const_pool.tile([128, NPAIR * 128], BF16)
        CQ = NPAIR * 128 // 4
        for q in range(4):
            nc.sync.dma_start(c4a_sb[:, CQ * q: CQ * (q + 1)],
                              c4a[:, CQ * q: CQ * (q + 1)])

        LA = 2
        state = {}
        stage = {}

        def s_tmm(i):
            """t' matmuls for pair i -> psum_t.  Config (128,128)."""
            pt = tpool.tile([128, 2, W], F32, tag="pt")
            for h in range(2):
                nc.tensor.matmul(
                    pt[:, h, :],
                    lhsT=c4a_sb[:, 128 * i: 128 * i + 128],
                    rhs=xn4_sb[:, h * W: h * W + W],
                    start=True, stop=True,
                    skip_group_check=True,
                )
            state[i] = {"pt": pt}

        def s_gen(i):
            """abs passes (ACT) + weight pass (DVE) for pair i."""
            st = state[i]
            pt = st["pt"]
            asb = apool.tile([128, 2, OWNW], BF16, tag="asb")
            nc.scalar.activation(
                asb[:, :, 0:512], pt[:],
                mybir.ActivationFunctionType.Abs,
            )
            nc.scalar.activation(
                asb[:, :, 512:512 + SW], pt[:, :, SOFF:SOFF + SW],
                mybir.ActivationFunctionType.Abs,
                bias=kb_sb[:], scale=1.0,
            )
            wt = wpool.tile([128, 2, OWNW], BF16, tag="wt")
            nc.vector.tensor_scalar(
                wt[:], asb[:], 1.0, 0.0,
                op0=mybir.AluOpType.subtract,
                op1=mybir.AluOpType.min,
            )
            st["wt"] = wt

        def mm_windows(po, lhs_of, wt, h, tile_pos):
            """7 band matmuls for one (row, plane-half) into psum po."""
            for k in range(KBLK):
                if k > 0:
                    rhs_acc = wt[:, h, 512 + GS[k] - SOFF: 512 + B[k] - SOFF]
                    nc.tensor.matmul(
                        po[:, GS[k]: B[k]], lhsT=lhs_of(k), rhs=rhs_acc,
                        start=False, stop=False,
                        tile_position=tile_pos,
                        skip_group_check=True,
                    )
                rhs_own = wt[:, h, B[k]: B[k + 1]]
                nc.tensor.matmul(
                    po[:, B[k]: B[k + 1]], lhsT=lhs_of(k), rhs=rhs_own,
                    start=True, stop=(k == KBLK - 1),
                    tile_position=tile_pos,
                    skip_group_check=True,
                )

        def s_mains(i):
            """main matmuls (config-grouped) + copies for pair i."""
            st = state.pop(i)
            wt = st["wt"]
            g, pr_in_g = i // 4, i % 4
            sidx = i % 2
            dte = state[("dte", g)]
            if sidx == 0:
                stage["o1"] = spool1.tile([128, 4, W], BF16, tag="o1", name="o1s")
                stage["o2"] = spool2.tile([128, 2, W], BF16, tag="o2", name="o2s")
            o1s, o2s = stage["o1"], stage["o2"]
            # ---- burst 1: config (128,128): planes 0-127, both rows ----
            po1 = opool1.tile([128, 2, W], F32, tag="po1")
            for h in range(2):
                base = (pr_in_g * 2 + h) * NPLANE
                mm_windows(po1[:, h, :], lambda k: dte[:, k, base: base + 128],
                           wt, h, None)
            # ---- burst 2: config (128,64): planes 128-191, pair-packed ----
            po2 = opool2.tile([128, W], F32, tag="po2")
            for h in range(2):
                base = (pr_in_g * 2 + h) * NPLANE
                mm_windows(po2[64 * h: 64 * h + 64],
                           lambda k: dte[:, k, base + 128: base + 192],
                           wt, h, (0, 64 * h))
            # ---- copies (osb2 alternates ACT/DVE for engine balance) ----
            nc.vector.tensor_scalar_mul(
                o1s[:, sidx * 2: sidx * 2 + 2, :], po1[:], -1.0)
            if i % 2 == 0:
                nc.scalar.mul(o2s[:, sidx, :], po2[:], -1.0)
            else:
                nc.vector.tensor_scalar_mul(o2s[:, sidx, :], po2[:], -1.0)
            if sidx == 1:
                y0 = (i - 1) * 2
                nc.sync.dma_start(out1[:, y0: y0 + 4, :], o1s[:])
                nc.sync.dma_start(
                    out2[:, :, i - 1: i + 1, :].rearrange("h p j x -> (h p) j x"),
                    o2s[:],
                )

        def s_imgdma(g):
            dte = dpool.tile([128, KBLK, YG * NPLANE], BF16, tag="dte")
            nc.sync.dma_start(dte[:], imgT[g].rearrange("k p f -> p k f"))
            state[("dte", g)] = dte

        s_imgdma(0)
        s_imgdma(1)
        s_imgdma(2)
        for i in range(NPAIR + LA):
            if i < NPAIR:
                if i % 4 == 0 and i // 4 + 3 < NG:
                    s_imgdma(i // 4 + 3)
            # mains first so the psum-freeing copies land at the head of the
            # ACT/DVE queues (ahead of the next pair's abs work)
            j = i - LA
            if j >= 0:
                s_mains(j)
                if j % 4 == 3:
                    state.pop(("dte", j // 4), None)
            if i < NPAIR:
                s_tmm(i)
            if 0 <= i - 1 < NPAIR:
                s_gen(i - 1)

    nc.compile()
    return nc


# ---------------- host-side helpers ----------------

def host_xn(c_u: np.ndarray) -> np.ndarray:
    """float64 reproduction of the reference displacement; xn [H, W] float64."""
    import math

    CUT = 16
    k = np.arange(1, CUT + 1, dtype=np.float64)
    i, j = np.meshgrid(k, k, indexing="ij")
    r = np.sqrt(i * i + j * j)
    e = (r < CUT + 0.5).astype(np.float64) / r
    x = np.linspace(0.0, 1.0, W, dtype=np.float64)
    s = np.sin(np.pi * x[:, None] * k[None, :])
    u = np.einsum("ij,xi,yj->yx", c_u.astype(np.float64) * e, s, s)
    Tw = 4.0 / (math.pi**3 * CUT**2 * math.log(CUT))
    dx = math.sqrt(Tw) * u * W
    xg = np.arange(W, dtype=np.float64)
    return np.clip(xg[None, :] - dx, 0.0, W - 1.0)


def _mask_bf16(v: np.ndarray) -> np.ndarray:
    """Truncate fp32 mantissa to bf16 (exact high piece)."""
    return (v.view(np.uint32) & np.uint32(0xFFFF0000)).view(np.float32)


def host_prep(img: np.ndarray, c_u: np.ndarray):
    """Build per-core input maps + window constants."""
    xn = host_xn(c_u)                   # [H, W] f64
    B, GS, SOFF, SW = compute_windows(xn)
    kown = np.zeros(W, np.float64)
    for k in range(1, KBLK):
        kown[B[k]:] = k
    xnp = (xn - 128.0 * kown[None, :]).astype(np.float32)   # xn'
    xh = _mask_bf16(xnp)
    r = (xnp - xh).astype(np.float32)
    xm = _mask_bf16(r)
    xl = (r - xm).astype(np.float32)

    planes = img.reshape(NPLANE, H, W)
    imgT_all = np.ascontiguousarray(
        planes.reshape(NPLANE, NCORES, NG, YG, W).transpose(1, 2, 4, 3, 0)
    ).astype(BF16_NP).reshape(NCORES, NG, KBLK, 128, YG * NPLANE)

    # c4a[p, 128*pr + m]: rows 4pr+{0,1,2} = 1, row 4pr+3 = -m, others 0
    c4m = np.zeros((128, NPAIR * 128), np.float32)
    marr = np.arange(128, dtype=np.float32)
    for pr in range(NPAIR):
        c4m[4 * pr + 0, 128 * pr: 128 * pr + 128] = 1.0
        c4m[4 * pr + 1, 128 * pr: 128 * pr + 128] = 1.0
        c4m[4 * pr + 2, 128 * pr: 128 * pr + 128] = 1.0
        c4m[4 * pr + 3, 128 * pr: 128 * pr + 128] = -marr
    c4m = c4m.astype(BF16_NP)

    in_maps = []
    for core in range(NCORES):
        xns = np.zeros((128, 2 * W), np.float32)
        for pr in range(NPAIR):
            for h in range(2):
                yg = core * YPC + 2 * pr + h
                xns[4 * pr + 0, h * W: h * W + W] = xh[yg]
                xns[4 * pr + 1, h * W: h * W + W] = xm[yg]
                xns[4 * pr + 2, h * W: h * W + W] = xl[yg]
                xns[4 * pr + 3, h * W: h * W + W] = 1.0
        in_maps.append({
            "imgT": imgT_all[core],
            "xn4": xns.astype(BF16_NP),
            "c4a": c4m,
            "kb": np.full((128, 1), -128.0, np.float32),
        })
    return in_maps, (B, GS, SOFF, SW)


def host_gather(outs: list) -> np.ndarray:
    """Assemble per-core out1 [128, YPC, W] + out2 [YPC, 64, W] -> [64,3,H,W]."""
    full = np.empty((NPLANE, H, W), np.float32)
    for core, om in enumerate(outs):
        sl = slice(core * YPC, (core + 1) * YPC)
        full[0:128, sl, :] = om["out1"].astype(np.float32)
        # out2 [h, pl, pr, x] -> rows 2*pr+h
        o2 = om["out2"].astype(np.float32).transpose(2, 0, 1, 3)  # [pr, h, pl, x]
        full[128:NPLANE, sl, :] = o2.reshape(YPC, 64, W).transpose(1, 0, 2)
    return full.reshape(64, 3, H, W)


# ---------------- harness entry point ----------------

_NC_CACHE = {}


def kernel(img: "np.ndarray", c_u: "np.ndarray", c_v: "np.ndarray") -> "np.ndarray":
    """Full-input entry: shard across 8 NeuronCores, run, reassemble."""
    img = np.ascontiguousarray(np.asarray(img, dtype=np.float32))
    c_u = np.asarray(c_u, dtype=np.float32)
    in_maps, wins = host_prep(img, c_u)
    key = tuple(wins[0]) + tuple(wins[1]) + (wins[2], wins[3])
    if _NC_CACHE.get("key") != key:
        _NC_CACHE["nc"] = build_program(*wins, num_devices=NCORES)
        _NC_CACHE["key"] = key
    from concourse.bass_utils import run_bass_kernel_spmd

    res = run_bass_kernel_spmd(
        _NC_CACHE["nc"], in_maps, core_ids=list(range(NCORES)), trace=False
    )
    return host_gather(res.results)


# revision 29
# speedup vs baseline: 1.0924x; 1.0924x over previous
"""Bass/Tile kernel for nn_Diffeo: horizontal bilinear remap as banded matmul.

v5 (bf16, config-uniform PE bursts): all I/O in bf16 (rel-err budget 2e-2;
measured ~6e-3).  Key perf insight from microbenchmarks: matmuls stream at
N/2.4GHz + 2.5ns ONLY while tile_size config stays constant; alternating
M=128 / M=64 matmuls forces a full pipeline drain per MM (~230ns).  So all
(128,128)-config MMs (t-gen + planes 0-127) are grouped per pair, then all
(128,64)-config MMs (planes 128-191, pair-packed via tile_position).

Per core (H-sharded, 64 rows y, all 192 b*c planes):
  out[bc, y, x_out] = sum_{x_in} imgT[y, x_in, bc] * hat(x_in - xn[y, x_out])

Weight generation per row pair with host-folded block offsets:
  xn'(x) = xn(x) - 128*kown(x)
  t'[p, x] = xn'(x) - p     K=128 matmul: lhsT = c4all slice (rows 4*pr+j
                            select pair pr's bf16-split pieces; other rows 0)
  abs1 (ACT):  a[x]     = |t'|        owned columns [0, 512)
  abs2 (ACT):  a[512+x] = |t' - 128|  accum strip (right-neighbor taps)
  pass2 (DVE): w = min(a - 1, 0) = -hat   (one 4x tensor_scalar, bf16)

Matmuls per (row, plane-half): 4 "own" (start=True, disjoint [B_k, B_k+1))
+ 3 "accum" (start=False, [GS_k, B_k)); accum for block k is issued BEFORE
own_k because own_k's start=True clears the bank's has_written bits.
Output staged 4 rows / 512KB+256KB DMAs; copies negate+cast on ACT/DVE.
"""

import sys
from contextlib import ExitStack

sys.path.insert(0, "/opt/trn_rl_repo")

import numpy as np
import ml_dtypes

import concourse.bass as bass
import concourse.mybir as mybir
import concourse.tile as tile
from concourse import bacc
from concourse._compat import axon_active

F32 = mybir.dt.float32
BF16 = mybir.dt.bfloat16
BF16_NP = ml_dtypes.bfloat16

H = W = 512
NPLANE = 192            # 64 batches * 3 channels
NCORES = 8
YPC = H // NCORES       # 64 rows per core
YG = 8                  # rows per input-DMA group
NG = YPC // YG          # 8 groups
KBLK = 4                # x_in blocks of 128
NPAIR = YPC // 2        # 32 row pairs per core


def compute_windows(xn: np.ndarray):
    """From the full xn field [H, W] (float64), derive the x_out windows."""
    mn = xn.min(axis=0)
    mx = xn.max(axis=0)
    tap_lo = np.floor(mn).astype(int)
    tap_hi = np.floor(mx).astype(int) + 1
    B = [0]
    GS = [0]
    for k in range(1, KBLK):
        lo = 128 * k
        cols_prev = np.nonzero(tap_lo < lo)[0]
        ge_prev = int(cols_prev.max()) + 1 if len(cols_prev) else 0
        ge_prev = min(ge_prev + 2, W)
        cols_k = np.nonzero(tap_hi >= lo)[0]
        gs_k = int(cols_k.min()) if len(cols_k) else W
        gs_k = max(gs_k - 2, 0)
        assert gs_k < ge_prev, f"no overlap at block {k}: {gs_k} {ge_prev}"
        assert ge_prev - 128 * (k - 1) < 256, "band too wide for scheme"
        B.append(ge_prev)
        GS.append(gs_k)
    B.append(W)
    assert all(B[i] < B[i + 1] for i in range(4)), f"bad B {B}"
    for k in range(1, KBLK):
        assert GS[k] >= B[k - 1], f"segment {k} leaks left: {GS[k]} < {B[k-1]}"
    SOFF = GS[1]
    SW = B[3] - SOFF
    SW = SW + (SW % 2)
    return B, GS, SOFF, SW


def build_program(B, GS, SOFF, SW, num_devices: int = NCORES):
    OWNW = 512 + SW
    nc = bacc.Bacc(
        "TRN2",
        target_bir_lowering=False,
        debug=not axon_active(),
        num_devices=num_devices,
    )
    imgT = nc.dram_tensor("imgT", [NG, KBLK, 128, YG * NPLANE], BF16,
                          kind="ExternalInput").ap()
    xn4 = nc.dram_tensor("xn4", [128, 2 * W], BF16, kind="ExternalInput").ap()
    c4a = nc.dram_tensor("c4a", [128, NPAIR * 128], BF16, kind="ExternalInput").ap()
    kb = nc.dram_tensor("kb", [128, 1], F32, kind="ExternalInput").ap()
    out1 = nc.dram_tensor("out1", [128, YPC, W], BF16, kind="ExternalOutput").ap()
    # out2 [h, plane64, pair, x]: per-partition runs are 2KB-contiguous
    out2 = nc.dram_tensor("out2", [2, 64, NPAIR, W], BF16, kind="ExternalOutput").ap()

    with tile.TileContext(nc) as tc, ExitStack() as ctx:
        const_pool = ctx.enter_context(tc.tile_pool(name="const", bufs=1))
        dpool = ctx.enter_context(tc.tile_pool(name="dt", bufs=4))
        tpool = ctx.enter_context(tc.tile_pool(name="psum_t", bufs=2, space="PSUM"))
        opool1 = ctx.enter_context(tc.tile_pool(name="psum_o1", bufs=2, space="PSUM"))
        opool2 = ctx.enter_context(tc.tile_pool(name="psum_o2", bufs=2, space="PSUM"))
        apool = ctx.enter_context(tc.tile_pool(name="abs", bufs=3))
        wpool = ctx.enter_context(tc.tile_pool(name="wt", bufs=4))
        spool1 = ctx.enter_context(tc.tile_pool(name="osb1", bufs=3))
        spool2 = ctx.enter_context(tc.tile_pool(name="osb2", bufs=3))

        xn4_sb = const_pool.tile([128, 2 * W], BF16)
        nc.sync.dma_start(xn4_sb[:], xn4[:])
        kb_sb = const_pool.tile([128, 1], F32)
        nc.sync.dma_start(kb_sb[:], kb[:])
        # split the 1MB c4a load so pair 0's slice lands quickly
        c4a_sb = const_pool.tile([128, NPAIR * 128], BF16)
        CQ = NPAIR * 128 // 4
        for q in range(4):
            nc.sync.dma_start(c4a_sb[:, CQ * q: CQ * (q + 1)],
                              c4a[:, CQ * q: CQ * (q + 1)])

        LA = 3
        state = {}
        stage = {}

        def s_tmm(i):
            """t' matmuls for pair i -> psum_t.  Config (128,128)."""
            pt = tpool.tile([128, 2, W], F32, tag="pt")
            for h in range(2):
                nc.tensor.matmul(
                    pt[:, h, :],
                    lhsT=c4a_sb[:, 128 * i: 128 * i + 128],
                    rhs=xn4_sb[:, h * W: h * W + W],
                    start=True, stop=True,
                    skip_group_check=True,
                )
            state[i] = {"pt": pt}

        def s_gen(i):
            """abs passes (ACT) + weight pass (DVE) for pair i."""
            st = state[i]
            pt = st["pt"]
            asb = apool.tile([128, 2, OWNW], BF16, tag="asb")
            nc.scalar.activation(
                asb[:, :, 0:512], pt[:],
                mybir.ActivationFunctionType.Abs,
            )
            nc.scalar.activation(
                asb[:, :, 512:512 + SW], pt[:, :, SOFF:SOFF + SW],
                mybir.ActivationFunctionType.Abs,
                bias=kb_sb[:], scale=1.0,
            )
            wt = wpool.tile([128, 2, OWNW], BF16, tag="wt")
            nc.vector.tensor_scalar(
                wt[:], asb[:], 1.0, 0.0,
                op0=mybir.AluOpType.subtract,
                op1=mybir.AluOpType.min,
            )
            st["wt"] = wt

        def mm_windows(po, lhs_of, wt, h, tile_pos):
            """7 band matmuls for one (row, plane-half) into psum po."""
            for k in range(KBLK):
                if k > 0:
                    rhs_acc = wt[:, h, 512 + GS[k] - SOFF: 512 + B[k] - SOFF]
                    nc.tensor.matmul(
                        po[:, GS[k]: B[k]], lhsT=lhs_of(k), rhs=rhs_acc,
                        start=False, stop=False,
                        tile_position=tile_pos,
                        skip_group_check=True,
                    )
                rhs_own = wt[:, h, B[k]: B[k + 1]]
                nc.tensor.matmul(
                    po[:, B[k]: B[k + 1]], lhsT=lhs_of(k), rhs=rhs_own,
                    start=True, stop=(k == KBLK - 1),
                    tile_position=tile_pos,
                    skip_group_check=True,
                )

        def s_mains(i):
            """main matmuls (config-grouped) + copies for pair i."""
            st = state.pop(i)
            wt = st["wt"]
            g, pr_in_g = i // 4, i % 4
            sidx = i % 2
            dte = state[("dte", g)]
            if sidx == 0:
                stage["o1"] = spool1.tile([128, 4, W], BF16, tag="o1", name="o1s")
                stage["o2"] = spool2.tile([128, 2, W], BF16, tag="o2", name="o2s")
            o1s, o2s = stage["o1"], stage["o2"]
            # ---- burst 1: config (128,128): planes 0-127, both rows;
            #      each row's copy issued right after its 7 MMs ----
            for h in range(2):
                base = (pr_in_g * 2 + h) * NPLANE
                po1 = opool1.tile([128, W], F32, tag="po1")
                mm_windows(po1, lambda k: dte[:, k, base: base + 128],
                           wt, h, None)
                nc.vector.tensor_scalar_mul(o1s[:, sidx * 2 + h, :],
                                            po1[:], -1.0)
            # ---- burst 2: config (128,64): planes 128-191, pair-packed ----
            po2 = opool2.tile([128, W], F32, tag="po2")
            for h in range(2):
                base = (pr_in_g * 2 + h) * NPLANE
                mm_windows(po2[64 * h: 64 * h + 64],
                           lambda k: dte[:, k, base + 128: base + 192],
                           wt, h, (0, 64 * h))
            if i % 2 == 0:
                nc.scalar.mul(o2s[:, sidx, :], po2[:], -1.0)
            else:
                nc.vector.tensor_scalar_mul(o2s[:, sidx, :], po2[:], -1.0)
            if sidx == 1:
                y0 = (i - 1) * 2
                nc.sync.dma_start(out1[:, y0: y0 + 4, :], o1s[:])
                nc.sync.dma_start(
                    out2[:, :, i - 1: i + 1, :].rearrange("h p j x -> (h p) j x"),
                    o2s[:],
                )

        def s_imgdma(g):
            dte = dpool.tile([128, KBLK, YG * NPLANE], BF16, tag="dte")
            nc.sync.dma_start(dte[:], imgT[g].rearrange("k p f -> p k f"))
            state[("dte", g)] = dte

        s_imgdma(0)
        s_imgdma(1)
        s_imgdma(2)
        for i in range(NPAIR + LA):
            if i < NPAIR:
                if i % 4 == 0 and i // 4 + 3 < NG:
                    s_imgdma(i // 4 + 3)
                # t-MM first: completes early in the step so abs/gen chain
                # for pair i starts with maximal slack
                s_tmm(i)
            j = i - LA
            if j >= 0:
                s_mains(j)
                if j % 4 == 3:
                    state.pop(("dte", j // 4), None)
            if 0 <= i - 1 < NPAIR:
                s_gen(i - 1)

    nc.compile()
    return nc


# ---------------- host-side helpers ----------------

def host_xn(c_u: np.ndarray) -> np.ndarray:
    """float64 reproduction of the reference displacement; xn [H, W] float64."""
    import math

    CUT = 16
    k = np.arange(1, CUT + 1, dtype=np.float64)
    i, j = np.meshgrid(k, k, indexing="ij")
    r = np.sqrt(i * i + j * j)
    e = (r < CUT + 0.5).astype(np.float64) / r
    x = np.linspace(0.0, 1.0, W, dtype=np.float64)
    s = np.sin(np.pi * x[:, None] * k[None, :])
    u = np.einsum("ij,xi,yj->yx", c_u.astype(np.float64) * e, s, s)
    Tw = 4.0 / (math.pi**3 * CUT**2 * math.log(CUT))
    dx = math.sqrt(Tw) * u * W
    xg = np.arange(W, dtype=np.float64)
    return np.clip(xg[None, :] - dx, 0.0, W - 1.0)


def _mask_bf16(v: np.ndarray) -> np.ndarray:
    """Truncate fp32 mantissa to bf16 (exact high piece)."""
    return (v.view(np.uint32) & np.uint32(0xFFFF0000)).view(np.float32)


def host_prep(img: np.ndarray, c_u: np.ndarray):
    """Build per-core input maps + window constants."""
    xn = host_xn(c_u)                   # [H, W] f64
    B, GS, SOFF, SW = compute_windows(xn)
    kown = np.zeros(W, np.float64)
    for k in range(1, KBLK):
        kown[B[k]:] = k
    xnp = (xn - 128.0 * kown[None, :]).astype(np.float32)   # xn'
    xh = _mask_bf16(xnp)
    r = (xnp - xh).astype(np.float32)
    xm = _mask_bf16(r)
    xl = (r - xm).astype(np.float32)

    planes = img.reshape(NPLANE, H, W)
    imgT_all = np.ascontiguousarray(
        planes.reshape(NPLANE, NCORES, NG, YG, W).transpose(1, 2, 4, 3, 0)
    ).astype(BF16_NP).reshape(NCORES, NG, KBLK, 128, YG * NPLANE)

    # c4a[p, 128*pr + m]: rows 4pr+{0,1,2} = 1, row 4pr+3 = -m, others 0
    c4m = np.zeros((128, NPAIR * 128), np.float32)
    marr = np.arange(128, dtype=np.float32)
    for pr in range(NPAIR):
        c4m[4 * pr + 0, 128 * pr: 128 * pr + 128] = 1.0
        c4m[4 * pr + 1, 128 * pr: 128 * pr + 128] = 1.0
        c4m[4 * pr + 2, 128 * pr: 128 * pr + 128] = 1.0
        c4m[4 * pr + 3, 128 * pr: 128 * pr + 128] = -marr
    c4m = c4m.astype(BF16_NP)

    in_maps = []
    for core in range(NCORES):
        xns = np.zeros((128, 2 * W), np.float32)
        for pr in range(NPAIR):
            for h in range(2):
                yg = core * YPC + 2 * pr + h
                xns[4 * pr + 0, h * W: h * W + W] = xh[yg]
                xns[4 * pr + 1, h * W: h * W + W] = xm[yg]
                xns[4 * pr + 2, h * W: h * W + W] = xl[yg]
                xns[4 * pr + 3, h * W: h * W + W] = 1.0
        in_maps.append({
            "imgT": imgT_all[core],
            "xn4": xns.astype(BF16_NP),
            "c4a": c4m,
            "kb": np.full((128, 1), -128.0, np.float32),
        })
    return in_maps, (B, GS, SOFF, SW)


def host_gather(outs: list) -> np.ndarray:
    """Assemble per-core out1 [128, YPC, W] + out2 [YPC, 64, W] -> [64,3,H,W]."""
    full = np.empty((NPLANE, H, W), np.float32)
    for core, om in enumerate(outs):
        sl = slice(core * YPC, (core + 1) * YPC)
        full[0:128, sl, :] = om["out1"].astype(np.float32)
        # out2 [h, pl, pr, x] -> rows 2*pr+h
        o2 = om["out2"].astype(np.float32).transpose(2, 0, 1, 3)  # [pr, h, pl, x]
        full[128:NPLANE, sl, :] = o2.reshape(YPC, 64, W).transpose(1, 0, 2)
    return full.reshape(64, 3, H, W)


# ---------------- harness entry point ----------------

_NC_CACHE = {}


def kernel(img: "np.ndarray", c_u: "np.ndarray", c_v: "np.ndarray") -> "np.ndarray":
    """Full-input entry: shard across 8 NeuronCores, run, reassemble."""
    img = np.ascontiguousarray(np.asarray(img, dtype=np.float32))
    c_u = np.asarray(c_u, dtype=np.float32)
    in_maps, wins = host_prep(img, c_u)
    key = tuple(wins[0]) + tuple(wins[1]) + (wins[2], wins[3])
    if _NC_CACHE.get("key") != key:
        _NC_CACHE["nc"] = build_program(*wins, num_devices=NCORES)
        _NC_CACHE["key"] = key
    from concourse.bass_utils import run_bass_kernel_spmd

    res = run_bass_kernel_spmd(
        _NC_CACHE["nc"], in_maps, core_ids=list(range(NCORES)), trace=False
    )
    return host_gather(res.results)


# revision 30
# speedup vs baseline: 1.1022x; 1.0090x over previous
"""Bass/Tile kernel for nn_Diffeo: horizontal bilinear remap as banded matmul.

v5 (bf16, config-uniform PE bursts): all I/O in bf16 (rel-err budget 2e-2;
measured ~6e-3).  Key perf insight from microbenchmarks: matmuls stream at
N/2.4GHz + 2.5ns ONLY while tile_size config stays constant; alternating
M=128 / M=64 matmuls forces a full pipeline drain per MM (~230ns).  So all
(128,128)-config MMs (t-gen + planes 0-127) are grouped per pair, then all
(128,64)-config MMs (planes 128-191, pair-packed via tile_position).

Per core (H-sharded, 64 rows y, all 192 b*c planes):
  out[bc, y, x_out] = sum_{x_in} imgT[y, x_in, bc] * hat(x_in - xn[y, x_out])

Weight generation per row pair with host-folded block offsets:
  xn'(x) = xn(x) - 128*kown(x)
  t'[p, x] = xn'(x) - p     K=128 matmul: lhsT = c4all slice (rows 4*pr+j
                            select pair pr's bf16-split pieces; other rows 0)
  abs1 (ACT):  a[x]     = |t'|        owned columns [0, 512)
  abs2 (ACT):  a[512+x] = |t' - 128|  accum strip (right-neighbor taps)
  pass2 (DVE): w = min(a - 1, 0) = -hat   (one 4x tensor_scalar, bf16)

Matmuls per (row, plane-half): 4 "own" (start=True, disjoint [B_k, B_k+1))
+ 3 "accum" (start=False, [GS_k, B_k)); accum for block k is issued BEFORE
own_k because own_k's start=True clears the bank's has_written bits.
Output staged 4 rows / 512KB+256KB DMAs; copies negate+cast on ACT/DVE.
"""

import sys
from contextlib import ExitStack

sys.path.insert(0, "/opt/trn_rl_repo")

import numpy as np
import ml_dtypes

import concourse.bass as bass
import concourse.mybir as mybir
import concourse.tile as tile
from concourse import bacc
from concourse._compat import axon_active

F32 = mybir.dt.float32
BF16 = mybir.dt.bfloat16
BF16_NP = ml_dtypes.bfloat16

H = W = 512
NPLANE = 192            # 64 batches * 3 channels
NCORES = 8
YPC = H // NCORES       # 64 rows per core
YG = 8                  # rows per input-DMA group
NG = YPC // YG          # 8 groups
KBLK = 4                # x_in blocks of 128
NPAIR = YPC // 2        # 32 row pairs per core


def compute_windows(xn: np.ndarray):
    """From the full xn field [H, W] (float64), derive the x_out windows."""
    mn = xn.min(axis=0)
    mx = xn.max(axis=0)
    tap_lo = np.floor(mn).astype(int)
    tap_hi = np.floor(mx).astype(int) + 1
    B = [0]
    GS = [0]
    for k in range(1, KBLK):
        lo = 128 * k
        cols_prev = np.nonzero(tap_lo < lo)[0]
        ge_prev = int(cols_prev.max()) + 1 if len(cols_prev) else 0
        ge_prev = min(ge_prev + 2, W)
        cols_k = np.nonzero(tap_hi >= lo)[0]
        gs_k = int(cols_k.min()) if len(cols_k) else W
        gs_k = max(gs_k - 2, 0)
        assert gs_k < ge_prev, f"no overlap at block {k}: {gs_k} {ge_prev}"
        assert ge_prev - 128 * (k - 1) < 256, "band too wide for scheme"
        B.append(ge_prev)
        GS.append(gs_k)
    B.append(W)
    assert all(B[i] < B[i + 1] for i in range(4)), f"bad B {B}"
    for k in range(1, KBLK):
        assert GS[k] >= B[k - 1], f"segment {k} leaks left: {GS[k]} < {B[k-1]}"
    SOFF = GS[1]
    SW = B[3] - SOFF
    SW = SW + (SW % 2)
    return B, GS, SOFF, SW


def build_program(B, GS, SOFF, SW, num_devices: int = NCORES):
    OWNW = 512 + SW
    nc = bacc.Bacc(
        "TRN2",
        target_bir_lowering=False,
        debug=not axon_active(),
        num_devices=num_devices,
    )
    imgT = nc.dram_tensor("imgT", [NG, KBLK, 128, YG * NPLANE], BF16,
                          kind="ExternalInput").ap()
    xn4 = nc.dram_tensor("xn4", [128, 2 * W], BF16, kind="ExternalInput").ap()
    c4a = nc.dram_tensor("c4a", [128, NPAIR * 128], BF16, kind="ExternalInput").ap()
    kb = nc.dram_tensor("kb", [128, 1], F32, kind="ExternalInput").ap()
    out1 = nc.dram_tensor("out1", [128, YPC, W], BF16, kind="ExternalOutput").ap()
    # out2 [h, plane64, pair, x]: per-partition runs are 2KB-contiguous
    out2 = nc.dram_tensor("out2", [2, 64, NPAIR, W], BF16, kind="ExternalOutput").ap()

    with tile.TileContext(nc) as tc, ExitStack() as ctx:
        const_pool = ctx.enter_context(tc.tile_pool(name="const", bufs=1))
        dpool = ctx.enter_context(tc.tile_pool(name="dt", bufs=4))
        tpool = ctx.enter_context(tc.tile_pool(name="psum_t", bufs=2, space="PSUM"))
        opool1 = ctx.enter_context(tc.tile_pool(name="psum_o1", bufs=2, space="PSUM"))
        opool2 = ctx.enter_context(tc.tile_pool(name="psum_o2", bufs=2, space="PSUM"))
        apool = ctx.enter_context(tc.tile_pool(name="abs", bufs=3))
        wpool = ctx.enter_context(tc.tile_pool(name="wt", bufs=4))
        spool1 = ctx.enter_context(tc.tile_pool(name="osb1", bufs=3))
        spool2 = ctx.enter_context(tc.tile_pool(name="osb2", bufs=3))

        xn4_sb = const_pool.tile([128, 2 * W], BF16)
        nc.sync.dma_start(xn4_sb[:], xn4[:])
        kb_sb = const_pool.tile([128, 1], F32)
        nc.sync.dma_start(kb_sb[:], kb[:])
        # split the 1MB c4a load so pair 0's slice lands quickly
        c4a_sb = const_pool.tile([128, NPAIR * 128], BF16)
        CQ = NPAIR * 128 // 4
        for q in range(4):
            nc.sync.dma_start(c4a_sb[:, CQ * q: CQ * (q + 1)],
                              c4a[:, CQ * q: CQ * (q + 1)])

        LA = 3
        state = {}
        stage = {}

        def s_tmm(i):
            """t' matmuls for pair i -> psum_t.  Config (128,128)."""
            pt = tpool.tile([128, 2, W], F32, tag="pt")
            for h in range(2):
                nc.tensor.matmul(
                    pt[:, h, :],
                    lhsT=c4a_sb[:, 128 * i: 128 * i + 128],
                    rhs=xn4_sb[:, h * W: h * W + W],
                    start=True, stop=True,
                    skip_group_check=True,
                )
            state[i] = {"pt": pt}

        def s_gen(i):
            """abs passes (ACT) + weight pass (DVE) for pair i."""
            st = state[i]
            pt = st["pt"]
            asb = apool.tile([128, 2, OWNW], BF16, tag="asb")
            nc.scalar.activation(
                asb[:, :, 0:512], pt[:],
                mybir.ActivationFunctionType.Abs,
            )
            nc.scalar.activation(
                asb[:, :, 512:512 + SW], pt[:, :, SOFF:SOFF + SW],
                mybir.ActivationFunctionType.Abs,
                bias=kb_sb[:], scale=1.0,
            )
            wt = wpool.tile([128, 2, OWNW], BF16, tag="wt")
            nc.vector.tensor_scalar(
                wt[:], asb[:], 1.0, 0.0,
                op0=mybir.AluOpType.subtract,
                op1=mybir.AluOpType.min,
            )
            st["wt"] = wt

        def mm_windows(po, lhs_of, wt, h, tile_pos):
            """7 band matmuls for one (row, plane-half) into psum po."""
            for k in range(KBLK):
                if k > 0:
                    rhs_acc = wt[:, h, 512 + GS[k] - SOFF: 512 + B[k] - SOFF]
                    nc.tensor.matmul(
                        po[:, GS[k]: B[k]], lhsT=lhs_of(k), rhs=rhs_acc,
                        start=False, stop=False,
                        tile_position=tile_pos,
                        skip_group_check=True,
                    )
                rhs_own = wt[:, h, B[k]: B[k + 1]]
                nc.tensor.matmul(
                    po[:, B[k]: B[k + 1]], lhsT=lhs_of(k), rhs=rhs_own,
                    start=True, stop=(k == KBLK - 1),
                    tile_position=tile_pos,
                    skip_group_check=True,
                )

        def s_mains(i):
            """main matmuls (config-grouped) + copies for pair i."""
            st = state.pop(i)
            wt = st["wt"]
            g, pr_in_g = i // 4, i % 4
            sidx = i % 2
            dte = state[("dte", g)]
            if sidx == 0:
                stage["o1"] = spool1.tile([128, 4, W], BF16, tag="o1", name="o1s")
                stage["o2"] = spool2.tile([128, 2, W], BF16, tag="o2", name="o2s")
            o1s, o2s = stage["o1"], stage["o2"]
            # ---- burst 1: config (128,128): planes 0-127, both rows;
            #      each row's copy issued right after its 7 MMs ----
            for h in range(2):
                base = (pr_in_g * 2 + h) * NPLANE
                po1 = opool1.tile([128, W], F32, tag="po1")
                mm_windows(po1, lambda k: dte[:, k, base: base + 128],
                           wt, h, None)
                nc.vector.tensor_scalar_mul(o1s[:, sidx * 2 + h, :],
                                            po1[:], -1.0)
            # ---- burst 2: config (128,64): planes 128-191, pair-packed ----
            po2 = opool2.tile([128, W], F32, tag="po2")
            for h in range(2):
                base = (pr_in_g * 2 + h) * NPLANE
                mm_windows(po2[64 * h: 64 * h + 64],
                           lambda k: dte[:, k, base + 128: base + 192],
                           wt, h, (0, 64 * h))
            if i % 2 == 0:
                nc.scalar.mul(o2s[:, sidx, :], po2[:], -1.0)
            else:
                nc.vector.tensor_scalar_mul(o2s[:, sidx, :], po2[:], -1.0)
            if sidx == 1:
                y0 = (i - 1) * 2
                nc.sync.dma_start(out1[:, y0: y0 + 4, :], o1s[:])
                nc.sync.dma_start(
                    out2[:, :, i - 1: i + 1, :].rearrange("h p j x -> (h p) j x"),
                    o2s[:],
                )

        def s_imgdma(g):
            dte = dpool.tile([128, KBLK, YG * NPLANE], BF16, tag="dte")
            nc.sync.dma_start(dte[:], imgT[g].rearrange("k p f -> p k f"))
            state[("dte", g)] = dte

        s_imgdma(0)
        s_imgdma(1)
        s_imgdma(2)
        for i in range(NPAIR + LA):
            if i < NPAIR:
                if i % 4 == 0 and i // 4 + 3 < NG:
                    s_imgdma(i // 4 + 3)
                # t-MM first: completes early in the step so abs/gen chain
                # for pair i starts with maximal slack
                s_tmm(i)
            # gen before mains: abs(i-1) must not queue behind the copies of
            # mains(j) on the ACT FIFO (those wait on the whole PE burst)
            if 0 <= i - 1 < NPAIR:
                s_gen(i - 1)
            j = i - LA
            if j >= 0:
                s_mains(j)
                if j % 4 == 3:
                    state.pop(("dte", j // 4), None)

    nc.compile()
    return nc


# ---------------- host-side helpers ----------------

def host_xn(c_u: np.ndarray) -> np.ndarray:
    """float64 reproduction of the reference displacement; xn [H, W] float64."""
    import math

    CUT = 16
    k = np.arange(1, CUT + 1, dtype=np.float64)
    i, j = np.meshgrid(k, k, indexing="ij")
    r = np.sqrt(i * i + j * j)
    e = (r < CUT + 0.5).astype(np.float64) / r
    x = np.linspace(0.0, 1.0, W, dtype=np.float64)
    s = np.sin(np.pi * x[:, None] * k[None, :])
    u = np.einsum("ij,xi,yj->yx", c_u.astype(np.float64) * e, s, s)
    Tw = 4.0 / (math.pi**3 * CUT**2 * math.log(CUT))
    dx = math.sqrt(Tw) * u * W
    xg = np.arange(W, dtype=np.float64)
    return np.clip(xg[None, :] - dx, 0.0, W - 1.0)


def _mask_bf16(v: np.ndarray) -> np.ndarray:
    """Truncate fp32 mantissa to bf16 (exact high piece)."""
    return (v.view(np.uint32) & np.uint32(0xFFFF0000)).view(np.float32)


def host_prep(img: np.ndarray, c_u: np.ndarray):
    """Build per-core input maps + window constants."""
    xn = host_xn(c_u)                   # [H, W] f64
    B, GS, SOFF, SW = compute_windows(xn)
    kown = np.zeros(W, np.float64)
    for k in range(1, KBLK):
        kown[B[k]:] = k
    xnp = (xn - 128.0 * kown[None, :]).astype(np.float32)   # xn'
    xh = _mask_bf16(xnp)
    r = (xnp - xh).astype(np.float32)
    xm = _mask_bf16(r)
    xl = (r - xm).astype(np.float32)

    planes = img.reshape(NPLANE, H, W)
    imgT_all = np.ascontiguousarray(
        planes.reshape(NPLANE, NCORES, NG, YG, W).transpose(1, 2, 4, 3, 0)
    ).astype(BF16_NP).reshape(NCORES, NG, KBLK, 128, YG * NPLANE)

    # c4a[p, 128*pr + m]: rows 4pr+{0,1,2} = 1, row 4pr+3 = -m, others 0
    c4m = np.zeros((128, NPAIR * 128), np.float32)
    marr = np.arange(128, dtype=np.float32)
    for pr in range(NPAIR):
        c4m[4 * pr + 0, 128 * pr: 128 * pr + 128] = 1.0
        c4m[4 * pr + 1, 128 * pr: 128 * pr + 128] = 1.0
        c4m[4 * pr + 2, 128 * pr: 128 * pr + 128] = 1.0
        c4m[4 * pr + 3, 128 * pr: 128 * pr + 128] = -marr
    c4m = c4m.astype(BF16_NP)

    in_maps = []
    for core in range(NCORES):
        xns = np.zeros((128, 2 * W), np.float32)
        for pr in range(NPAIR):
            for h in range(2):
                yg = core * YPC + 2 * pr + h
                xns[4 * pr + 0, h * W: h * W + W] = xh[yg]
                xns[4 * pr + 1, h * W: h * W + W] = xm[yg]
                xns[4 * pr + 2, h * W: h * W + W] = xl[yg]
                xns[4 * pr + 3, h * W: h * W + W] = 1.0
        in_maps.append({
            "imgT": imgT_all[core],
            "xn4": xns.astype(BF16_NP),
            "c4a": c4m,
            "kb": np.full((128, 1), -128.0, np.float32),
        })
    return in_maps, (B, GS, SOFF, SW)


def host_gather(outs: list) -> np.ndarray:
    """Assemble per-core out1 [128, YPC, W] + out2 [YPC, 64, W] -> [64,3,H,W]."""
    full = np.empty((NPLANE, H, W), np.float32)
    for core, om in enumerate(outs):
        sl = slice(core * YPC, (core + 1) * YPC)
        full[0:128, sl, :] = om["out1"].astype(np.float32)
        # out2 [h, pl, pr, x] -> rows 2*pr+h
        o2 = om["out2"].astype(np.float32).transpose(2, 0, 1, 3)  # [pr, h, pl, x]
        full[128:NPLANE, sl, :] = o2.reshape(YPC, 64, W).transpose(1, 0, 2)
    return full.reshape(64, 3, H, W)


# ---------------- harness entry point ----------------

_NC_CACHE = {}


def kernel(img: "np.ndarray", c_u: "np.ndarray", c_v: "np.ndarray") -> "np.ndarray":
    """Full-input entry: shard across 8 NeuronCores, run, reassemble."""
    img = np.ascontiguousarray(np.asarray(img, dtype=np.float32))
    c_u = np.asarray(c_u, dtype=np.float32)
    in_maps, wins = host_prep(img, c_u)
    key = tuple(wins[0]) + tuple(wins[1]) + (wins[2], wins[3])
    if _NC_CACHE.get("key") != key:
        _NC_CACHE["nc"] = build_program(*wins, num_devices=NCORES)
        _NC_CACHE["key"] = key
    from concourse.bass_utils import run_bass_kernel_spmd

    res = run_bass_kernel_spmd(
        _NC_CACHE["nc"], in_maps, core_ids=list(range(NCORES)), trace=False
    )
    return host_gather(res.results)
